# revision 1
# baseline (speedup 1.0000x reference)
"""Trainium2 Bass kernel for windowed cross-attention (Swin-style).

Problem (hardcoded): b=256 windows, nq=256 queries, n=576 keys, DIM=192,
HEADS=6, hd=32, relative-position bias table (1521, 6) gathered by rpi.

Sharding: pure data parallel over the leading window axis b across 8 cores
(32 windows/core).  Weights + gathered bias are replicated.

Per-core dataflow (matmul operands bf16, fp32 PSUM accumulate):
  - x_q / x_kv loaded via HWDGE DMA-transpose so the contract (feature) dim
    lands on SBUF partitions.
  - projections: qT [192, 256], kT [192, 576], v' [576, head, 33] (col 32
    is ones -> softmax denominator comes free out of the PV matmul).
  - per (head, n-chunk): attnT [n_chunk, 256] = kT_h.T @ qT_h  (K=32 matmuls
    at rotating row-group base partitions -> PE sub-array concurrency).
    PSUM outputs are placed at full-bank strides (matmul outputs must be
    bank-aligned on this toolchain).
  - exp on ScalarE (attention scale folded into the activation scale; logits
    are bounded ~0.5 so no max subtraction), multiply by host-precomputed
    exp(bias) on VectorE.
  - PV transposed: out'_h [33, 256] = v'_h.T @ E_h accumulated over n-chunks,
    two heads col-tiled per PSUM tile (partitions 0:33 / 64:97).
  - unnormalized out' + sumexp are DMA'd out; the divide + head transpose
    happen on the host during unsharding.
"""

import numpy as np
import ml_dtypes

# ---------------- problem constants (hardcoded per contract) ----------------
B = 256
NQ = 256
N = 576
DIM = 192
HEADS = 6
HD = 32
NCORES = 8
BW = B // NCORES          # windows per core = 32
NPAIRS = BW // 2          # dma-transpose batches 2 windows
NCH = 5                   # n chunks: 4x128 + 64
SCALE = HD ** -0.5

BF16 = ml_dtypes.bfloat16

_CACHE = {}


def _build_bass(npairs=NPAIRS, split_waits=True):
    import concourse.bass as bass
    import concourse.mybir as mybir
    import concourse.tile as tile

    fp32 = mybir.dt.float32
    bf16 = mybir.dt.bfloat16
    Exp = mybir.ActivationFunctionType.Exp

    nc = bass.Bass()

    bw = 2 * npairs
    xq = nc.declare_dram_parameter("xq", [bw, NQ, DIM], bf16, isOutput=False)
    xkv = nc.declare_dram_parameter("xkv", [bw, N, DIM], bf16, isOutput=False)
    # weight chunks: [:, 0, :] = rows 0:128 of W^T; [64:128, 1, :] = rows
    # 128:192 of W^T (placed at partitions 64:128 so lhsT/rhs base
    # partitions match for the second K-chunk)
    wq = nc.declare_dram_parameter("wq", [2, 128, DIM], bf16, isOutput=False)
    wk = nc.declare_dram_parameter("wk", [2, 128, DIM], bf16, isOutput=False)
    wv = nc.declare_dram_parameter("wv", [2, 128, DIM], bf16, isOutput=False)
    # exp(rpb) arranged [chunk, n_row_in_chunk, head*256 + q]
    expb = nc.declare_dram_parameter(
        "expb", [NCH, 128, HEADS * NQ], bf16, isOutput=False)
    # unnormalized out'^T: [w, pair, 2*33, q]; rows 0:33 head 2p, 33:66
    # head 2p+1; row 32/65 = sumexp
    out = nc.declare_dram_parameter("out", [bw, 3, 66, NQ], bf16, isOutput=True)

    with tile.TileContext(nc) as tc:
        with (
            tc.tile_pool(name="const", bufs=1) as const,
            tc.tile_pool(name="xin", bufs=3) as xin,
            tc.tile_pool(name="proj", bufs=3) as proj,
            tc.tile_pool(name="esb", bufs=3) as esb,
            tc.tile_pool(name="osb", bufs=4) as osb,
            tc.tile_pool(name="qk", bufs=2, space="PSUM") as qk_pool,
            tc.tile_pool(name="pps", bufs=2, space="PSUM") as pps,
            tc.tile_pool(name="pvp", bufs=2, space="PSUM") as pv_pool,
        ):
            # ---- constants ----
            wq_sb = const.tile([128, 2, DIM], bf16, tag="wq")
            wk_sb = const.tile([128, 2, DIM], bf16, tag="wk")
            wv_sb = const.tile([128, 2, DIM], bf16, tag="wv")
            expb_sb = const.tile([128, NCH, HEADS * NQ], bf16, tag="expb")
            zrow = const.tile([1, 256], bf16, tag="zrow")
            scr_d = const.tile([1, 8], bf16, tag="scr_d")
            scr_a = const.tile([1, 8], bf16, tag="scr_a")
            nc.sync.dma_start(out=wq_sb, in_=wq.rearrange("c p d -> p c d"))
            nc.sync.dma_start(out=wk_sb, in_=wk.rearrange("c p d -> p c d"))
            nc.sync.dma_start(out=wv_sb, in_=wv.rearrange("c p d -> p c d"))
            nc.sync.dma_start(out=expb_sb, in_=expb.rearrange("c p d -> p c d"))
            nc.vector.memset(zrow, 0.0)

            prev = {"e2": None, "o": None}

            def absorb(*aps):
                # tiny LDWEIGHTS ops that consume APs so cross-engine/DMA
                # waits land on them instead of on matmuls
                for a in aps:
                    if a.dtype == fp32:
                        a = a.bitcast(bf16)[:, 0:1]
                    nc.tensor.ldweights(a)

            def dve_absorb(a):
                if a.dtype == fp32:
                    a = a.bitcast(bf16)
                nc.vector.tensor_copy(out=scr_d[0:1, 0:1], in_=a[0:1, 0:1])

            def act_absorb(a):
                if a.dtype == fp32:
                    a = a.bitcast(bf16)
                nc.scalar.copy(out=scr_a[0:1, 0:1], in_=a[0:1, 0:1])

            for pair in range(npairs):
                # ---- input loads (2 windows, transposed) ----
                xqt_a = xin.tile([128, 2, NQ], bf16, tag="xqt_a")   # k 0:128
                xqt_b = xin.tile([128, 2, NQ], bf16, tag="xqt_b")   # k 64:192
                xkt_a = xin.tile([128, 2, N], bf16, tag="xkt_a")
                xkt_b = xin.tile([128, 2, N], bf16, tag="xkt_b")
                w0 = 2 * pair
                nc.sync.dma_start_transpose(
                    out=xqt_a.rearrange("p a b -> p (a b)"),
                    in_=xq[w0:w0 + 2, :, 0:128].rearrange("a b c -> (a b) c"))
                nc.sync.dma_start_transpose(
                    out=xqt_b.rearrange("p a b -> p (a b)"),
                    in_=xq[w0:w0 + 2, :, 64:192].rearrange("a b c -> (a b) c"))
                nc.sync.dma_start_transpose(
                    out=xkt_a.rearrange("p a b -> p (a b)"),
                    in_=xkv[w0:w0 + 2, :, 0:128].rearrange("a b c -> (a b) c"))
                nc.sync.dma_start_transpose(
                    out=xkt_b.rearrange("p a b -> p (a b)"),
                    in_=xkv[w0:w0 + 2, :, 64:192].rearrange("a b c -> (a b) c"))


                # ---- q/k projections, batched over the 2 windows (shared
                # weight loads, N=512).  qT [96, 2, 256], kT [96, 2, 576];
                # M-chunks of 96 (heads 0-2 / 3-5) so per-head row groups
                # stay at base partitions {0, 32, 64}.
                qt = [proj.tile([96, 2, NQ], bf16, tag=f"qt{i}", name=f"qt{i}")
                      for i in range(2)]
                kt = [proj.tile([96, 2, N], bf16, tag=f"kt{i}", name=f"kt{i}")
                      for i in range(2)]
                xqa2 = xqt_a.rearrange("p a b -> p (a b)")
                xqb2 = xqt_b.rearrange("p a b -> p (a b)")
                for mi in range(2):
                    oc = slice(96 * mi, 96 * mi + 96)
                    ps = pps.tile([128, 512], fp32, tag="pps")
                    nc.tensor.matmul(ps[0:96, :], wq_sb[:, 0, oc], xqa2,
                                     start=True, stop=False)
                    nc.tensor.matmul(ps[0:96, :], wq_sb[64:128, 1, oc],
                                     xqb2[64:128, :], start=False, stop=True)
                    nc.scalar.copy(
                        out=qt[mi].rearrange("p a b -> p (a b)"),
                        in_=ps[0:96, :])
                    absorb(qt[mi][0:1, 0, 0:1])
                    for nci in range(3):
                        nw = 256 if nci < 2 else 64
                        ncols = slice(256 * nci, 256 * nci + nw)
                        ps = pps.tile([128, 512], fp32, tag="pps")
                        nc.tensor.matmul(ps[0:96, 0:2 * nw],
                                         wk_sb[:, 0, oc],
                                         xkt_a[:, :, ncols],
                                         start=True, stop=False)
                        nc.tensor.matmul(ps[0:96, 0:2 * nw],
                                         wk_sb[64:128, 1, oc],
                                         xkt_b[64:128, :, ncols],
                                         start=False, stop=True)
                        nc.vector.tensor_copy(
                            out=kt[mi][:, :, ncols],
                            in_=ps[0:96, 0:2 * nw].rearrange(
                                "p (a b) -> p a b", a=2))

                for ws in range(2):
                    w = w0 + ws
                    xka, xkb = xkt_a[:, ws, :], xkt_b[:, ws, :]

                    # ---- v projection: v' [n_chunk, head, 33] (col 32 = 1)
                    vsb = proj.tile([128, NCH, HEADS, 34], bf16, tag="vsb")
                    for c in range(NCH):
                        rows = 128 if c < 4 else 64
                        cs = slice(128 * c, 128 * c + rows)
                        ps = pps.tile([128, 288], fp32, tag="pps")
                        nc.tensor.matmul(ps[0:rows, 0:DIM], xka[:, cs],
                                         wv_sb[:, 0, :], start=True, stop=False)
                        nc.tensor.matmul(ps[0:rows, 0:DIM], xkb[64:128, cs],
                                         wv_sb[64:128, 1, :],
                                         start=False, stop=True)
                        nc.vector.memset(vsb[0:rows, c, :, 32], 1.0)
                        nc.vector.tensor_copy(
                            out=vsb[0:rows, c, :, 0:32],
                            in_=ps[0:rows, 0:DIM].rearrange(
                                "p (h d) -> p h d", h=HEADS))

                    # ---- QK^T + exp + bias, per n-chunk ----
                    e_sb = esb.tile([128, NCH, HEADS * NQ], bf16, tag="e")
                    e2_sb = esb.tile([128, NCH, HEADS * NQ], bf16, tag="e2")
                    for c in range(NCH):
                        rows = 128 if c < 4 else 64
                        cs = slice(128 * c, 128 * c + rows)
                        # heads at 512-col strides: matmul outputs
                        # bank-aligned; 2-head passes (4 banks total) free
                        # PSUM for pps/pv double-buffering
                        for hp2 in range(3):
                            qkp = qk_pool.tile([128, 2, 512], fp32, tag="qk",
                                               name=f"qk{hp2}")
                            for hi in range(2):
                                h = 2 * hp2 + hi
                                hh = slice(32 * (h % 3), 32 * (h % 3) + 32)
                                kth, qth = (kt[0], qt[0]) if h < 3 else                                     (kt[1], qt[1])
                                nc.tensor.matmul(
                                    qkp[0:rows, hi, 0:NQ],
                                    kth[hh, ws, cs], qth[hh, ws, :],
                                    start=True, stop=True)
                            # evacuate: exp(scale*logit) then * exp(bias)
                            nc.scalar.activation(
                                out=e_sb[0:rows, c,
                                         512 * hp2:512 * hp2 + 512].rearrange(
                                    "p (h q) -> p h q", h=2),
                                in_=qkp[0:rows, :, 0:NQ],
                                func=Exp, scale=float(SCALE))
                        nc.vector.tensor_mul(
                            e2_sb[0:rows, c, :], e_sb[0:rows, c, :],
                            expb_sb[0:rows, c, :])

                    # ---- PV transposed: out'_h [33, 256], 2 heads/tile ----
                    o_sb = osb.tile([128, 3, NQ], bf16, tag="o")
                    for p in range(3):
                        pvps = pv_pool.tile([128, NQ], fp32, tag="pv")
                        # zeroing matmul claims the bank's has_written bits
                        nc.tensor.matmul(
                            pvps, zrow[:, 0:128], zrow[:, 0:NQ],
                            start=True, stop=False, skip_group_check=True)
                        for c in range(NCH):
                            rows = 128 if c < 4 else 64
                            for hi in range(2):
                                h = 2 * p + hi
                                nc.tensor.matmul(
                                    pvps[64 * hi:64 * hi + 33, :],
                                    vsb[0:rows, c, h, 0:33],
                                    e2_sb[0:rows, c, NQ * h:NQ * h + NQ],
                                    start=False,
                                    stop=(c == NCH - 1 and hi == 1),
                                    skip_group_check=True)
                        nc.vector.tensor_copy(out=o_sb[0:33, p, :],
                                              in_=pvps[0:33, :])
                        nc.vector.tensor_copy(out=o_sb[64:97, p, :],
                                              in_=pvps[64:97, :])
                    nc.sync.dma_start(
                        out=out[w][:, 0:33, :].rearrange("a p b -> p a b"),
                        in_=o_sb[0:33])
                    nc.sync.dma_start(
                        out=out[w][:, 33:66, :].rearrange("a p b -> p a b"),
                        in_=o_sb[64:97])
                    prev["e2"] = e2_sb
                    prev["o"] = o_sb

    if split_waits:
        _split_multi_waits(nc, mybir)
    return nc


_NO_SPLIT_OPCODES = {
    "UnconditionalBranch", "Call", "ISA", "CompareAndBranch", "BranchHint",
    "Halt", "IndirectBranch",
}


def _split_multi_waits(nc, mybir):
    """Walrus ISA structs accept a single sync wait per instruction; hoist
    extras onto preceding same-engine NoOps (sequencer waits)."""
    k = 0
    for f in nc.m.functions:
        for bb in f.blocks:
            il = bb.instructions
            new = []
            for inst in il:
                si = inst.sync_info
                ow = list(si.on_wait) if si is not None and si.on_wait else []
                if len(ow) > 1 and inst.concise_opcode not in _NO_SPLIT_OPCODES:
                    for wslot in ow[:-1]:
                        k += 1
                        new.append(mybir.InstNoOp(
                            name=f"hoistw-{k}",
                            engine=inst.engine,
                            sync_info=mybir.SyncInfo(
                                on_wait=[wslot], on_update=[]),
                        ))
                    inst.sync_info = mybir.SyncInfo(
                        on_wait=[ow[-1]], on_update=list(si.on_update))
                new.append(inst)
            bb.instructions = new


def _prepare_shared(Wq, Wkv, rpi, bias_table):
    """Host-side constant prep (replicated across cores)."""
    Wq = np.asarray(Wq, np.float32)
    Wkv = np.asarray(Wkv, np.float32)
    bias_table = np.asarray(bias_table, np.float32)

    def chunks(WT):
        a = np.ascontiguousarray(WT[0:128]).astype(BF16)
        bpad = np.zeros((128, DIM), np.float32)
        bpad[64:128] = WT[128:192]
        return np.stack([a, bpad.astype(BF16)])

    wq_c = chunks(Wq.T)
    wk_c = chunks(Wkv[:DIM].T)
    wv_c = chunks(Wkv[DIM:].T)

    rpb = bias_table[np.asarray(rpi, np.int64).ravel()].reshape(NQ, N, HEADS)
    arr = np.exp(rpb.transpose(2, 1, 0).astype(np.float32))   # (h, n, q)
    expb = np.zeros((NCH, 128, HEADS * NQ), np.float32)
    for c in range(NCH):
        rows = 128 if c < 4 else 64
        for h in range(HEADS):
            expb[c, :rows, h * NQ:(h + 1) * NQ] = \
                arr[h, 128 * c:128 * c + rows, :]
    return wq_c, wk_c, wv_c, expb.astype(BF16)


def _postprocess(raw):
    """raw: (b, 3, 66, 256) unnormalized out'^T + sumexp -> (b, 256, 192)."""
    b = raw.shape[0]
    r = raw.astype(np.float32).reshape(b, 3, 2, 33, NQ)
    num = r[:, :, :, 0:32, :]                  # (b, 3, 2, 32, q)
    den = r[:, :, :, 32:33, :]                 # (b, 3, 2, 1, q)
    o = num / den
    # head h = 2*p + hi -> out[:, q, 32*h + d]
    o = o.transpose(0, 4, 1, 2, 3)             # (b, q, 3, 2, 32)
    return np.ascontiguousarray(o.reshape(b, NQ, DIM), dtype=np.float32)


def kernel(x_q, x_kv, rpi, Wq, Wkv, bias_table):
    from concourse.bass_utils import run_bass_kernel_spmd

    if "nc" not in _CACHE:
        _CACHE["nc"] = _build_bass()
    nc = _CACHE["nc"]

    wq_c, wk_c, wv_c, expb = _prepare_shared(Wq, Wkv, rpi, bias_table)

    xq_bf = np.asarray(x_q, np.float32).astype(BF16)
    xkv_bf = np.asarray(x_kv, np.float32).astype(BF16)

    in_maps = []
    for i in range(NCORES):
        sl = slice(i * BW, (i + 1) * BW)
        in_maps.append({
            "xq": np.ascontiguousarray(xq_bf[sl]),
            "xkv": np.ascontiguousarray(xkv_bf[sl]),
            "wq": wq_c, "wk": wk_c, "wv": wv_c, "expb": expb,
        })

    res = run_bass_kernel_spmd(nc, in_maps, core_ids=list(range(NCORES)))
    out = np.concatenate(
        [_postprocess(res.results[i]["out"]) for i in range(NCORES)], axis=0)
    return out

